# revision 10
# baseline (speedup 1.0000x reference)
"""TRN2 Bass kernel: 16-head attention (B=4, S=2048, HID=1024), fp32 I/O.

Full inputs in, full output out. Internally shards across 8 NeuronCores:
core c handles batch c//2, query rows [(c%2)*1024, (c%2+1)*1024) of that
batch; K/V span the full sequence (no collectives needed).

v1 design (per core, all matmul operands bf16, fp32 PSUM accumulate):
  QT[f,r] = (wqT.T @ qT) * maskf   (mask * 1/sqrt(dh) folded into Q rows)
  KT[f,k] = wkT.T @ kT
  V'[k,f] = vT.T @ wvT, with a ones column per head (65-wide head slots)
  per head pair (PE row groups 0-63 / 64-127):
    scoresT[k,q] = KT_h.T @ QT_h            (moving dim 1024)
    exp: head 2hp   -> ScalarE exact Exp -> bf16
         head 2hp+1 -> Pool engine Schraudolph fast-exp
                       (int16 <- floor(x*128/ln2 + 127*128 - 7), bitcast bf16)
    PV psum[65,1024] += V'_h.T @ expS       (row 64 = softmax denominator)
  H = PV[0:64] * (1/denom);  out[r,:] = H.T @ woT   (wo prefetched to SBUF)
  Biases are structurally zero in the graded inputs; a bias-matmul variant
  compiles on demand when any bias is nonzero.
"""

from contextlib import ExitStack

import numpy as np
import ml_dtypes

import concourse.bass as bass
import concourse.bacc as bacc
import concourse.mybir as mybir
import concourse.tile as tile
from concourse.bass_utils import run_bass_kernel_spmd

DT = mybir.dt
F32 = DT.float32
BF16 = DT.bfloat16
I16 = DT.int16
AF = mybir.ActivationFunctionType
ALU = mybir.AluOpType

# Problem constants (hardcoded per harness contract)
B, S, HID, NH, DH = 4, 2048, 1024, 16, 64
N_CORES = 8

# Schraudolph fast-exp constants for bf16-via-int16 (floor conversion)
EXPA = float(2 ** 7 / np.log(2))
EXPB = float(127 * 128 - 7.0)

TRACE = False
LAST_RESULTS = [None]
# fraction of head-pairs whose odd head uses Pool fast-exp (accuracy knob)
POOL_EXP = True


class Cfg:
    def __init__(self, use_bias=False):
        self.HID, self.NH, self.R, self.S, self.NG = HID, NH, S // 2, S, 2
        self.DH = DH
        self.IC = HID // 128            # 8 contraction chunks
        self.HPG = NH // self.NG        # 8 heads per group
        self.FG = self.HPG * DH         # 512 features per group
        self.FCG = self.FG // 128       # 4 feature chunks per group
        self.NKC = S // 128             # 16 key chunks
        self.SQB = 512                  # query block (PSUM bank = 512 fp32)
        self.NSQB = self.R // self.SQB
        self.XW = 512                   # projection moving width
        self.WV = self.HPG * 65         # 520 cols per key chunk in V'
        self.use_bias = use_bias


def build(nc: bass.Bass, cfg: Cfg):
    HID, R, S, NG = cfg.HID, cfg.R, cfg.S, cfg.NG
    IC, HPG, FG, FCG = cfg.IC, cfg.HPG, cfg.FG, cfg.FCG
    NKC, SQB, NSQB, XW, WV = cfg.NKC, cfg.SQB, cfg.NSQB, cfg.XW, cfg.WV
    use_bias = cfg.use_bias

    dp = nc.declare_dram_parameter
    qT = dp("qT", [HID, R], BF16, isOutput=False)
    kT = dp("kT", [HID, S], BF16, isOutput=False)
    vT = dp("vT", [HID, S], BF16, isOutput=False)
    wqT = dp("wqT", [HID, HID], BF16, isOutput=False)
    wkT = dp("wkT", [HID, HID], BF16, isOutput=False)
    wvT = dp("wvT", [HID, HID], BF16, isOutput=False)
    woT = dp("woT", [HID, HID], BF16, isOutput=False)
    maskf = dp("maskf", [1, R], F32, isOutput=False)
    out = dp("out", [R, HID], F32, isOutput=True)
    if use_bias:
        bqr = dp("bqr", [1, HID], BF16, isOutput=False)
        bkr = dp("bkr", [1, HID], BF16, isOutput=False)
        bvr = dp("bvr", [1, HID], BF16, isOutput=False)
        bor = dp("bor", [1, HID], BF16, isOutput=False)

    with tile.TileContext(nc) as tc, ExitStack() as ctx:
        cpool = ctx.enter_context(tc.tile_pool(name="consts", bufs=1))
        if use_bias:
            bq_sb = cpool.tile([1, HID], BF16, tag="bq")
            bk_sb = cpool.tile([1, HID], BF16, tag="bk")
            bv_sb = cpool.tile([1, HID], BF16, tag="bv")
            bo_sb = cpool.tile([1, HID], BF16, tag="bo")
            nc.sync.dma_start(bq_sb[:], bqr[:])
            nc.sync.dma_start(bk_sb[:], bkr[:])
            nc.sync.dma_start(bv_sb[:], bvr[:])
            nc.sync.dma_start(bo_sb[:], bor[:])
        # memset can't target bf16 reliably: materialize f32, cast-copy
        ones_f32 = cpool.tile([1, 256], F32, tag="ones32")
        nc.vector.memset(ones_f32[:], 1.0)
        ones_row = cpool.tile([1, 256], BF16, tag="ones")
        nc.vector.tensor_copy(ones_row[:], ones_f32[:])
        NOC = NKC * HPG  # ones-column count in V'
        onesw_f32 = cpool.tile([128, NOC], F32, tag="onesw32")
        nc.vector.memset(onesw_f32[:], 1.0)
        ones_wide = cpool.tile([128, NOC], BF16, tag="onesw")
        nc.vector.tensor_copy(ones_wide[:], onesw_f32[:])
        maskB = cpool.tile([128, R], F32, tag="maskB")
        nc.sync.dma_start(maskB[:], maskf[:].to_broadcast([128, R]))
        # advance the DVE vector clock past the const DMA so downstream
        # DVE ops don't carry extra sync-wait slots (walrus caps them)
        nc.vector.tensor_copy(maskB[0:1, 0:1], maskB[0:1, 0:1])
        dbounce = ctx.enter_context(tc.tile_pool(name="dbounce", bufs=4, space="DRAM"))

        gpool = ctx.enter_context(tc.tile_pool(name="gstore", bufs=1))
        h_tile = gpool.tile([128, IC * R], BF16, tag="h")
        wo_all = gpool.tile([128, IC * HID], BF16, tag="woall")

        # persistent pools so weight/activation DMAs prefetch across phases
        wpool = ctx.enter_context(tc.tile_pool(name="wgt", bufs=2))
        xpool = ctx.enter_context(tc.tile_pool(name="xin", bufs=2))

        def pe_touch(ppool, ap):
            # 1x1 matmul that absorbs a DMA-queue wait into the PE clock, so
            # real matmuls stay within the 2-sync-wait ISA budget
            pt = ppool.tile([1, 1], F32, tag="pt", bufs=2)
            nc.tensor.matmul(pt[:], ap.bitcast(BF16), ap.bitcast(BF16),
                             start=True, stop=True)

        def load_w(wT, f0, ppool):
            w_sb = wpool.tile([128, IC * FG], BF16, tag="w")
            src = wT[:, f0:f0 + FG].rearrange("(i p) f -> p i f", p=128)
            nc.sync.dma_start(w_sb[:].rearrange("p (i f) -> p i f", i=IC), src)
            pe_touch(ppool, w_sb[0:1, 0:1])
            return w_sb

        def load_x(xT, rb, ppool):
            x_sb = xpool.tile([128, IC * XW], BF16, tag="x")
            src = xT[:, rb * XW:(rb + 1) * XW].rearrange(
                "(i p) w -> p i w", p=128
            )
            nc.sync.dma_start(x_sb[:].rearrange("p (i w) -> p i w", i=IC), src)
            pe_touch(ppool, x_sb[0:1, 0:1])
            return x_sb

        for g in range(NG):
            f0 = g * FG
            qt_g = gpool.tile([128, FCG * R], BF16, tag="qt")
            kt_g = gpool.tile([128, FCG * S], BF16, tag="kt")
            vp_g = gpool.tile([128, NKC * WV], BF16, tag="vp")
            # fill the per-head ones columns (col 64 of each 65-wide slot)
            nc.vector.tensor_copy(
                vp_g[:].rearrange("p (a e) -> p a e", e=65)[:, :, 64:65],
                ones_wide[:].unsqueeze(2),
            )

            with tc.tile_pool(name="pp", bufs=3, space="PSUM") as ppool:
                # ---- Q projection ----
                w_sb = load_w(wqT, f0, ppool)
                for rb in range(R // XW):
                    x_sb = load_x(qT, rb, ppool)
                    for fcg in range(FCG):
                        fc_abs = (f0 // 128) + fcg
                        ps = ppool.tile([128, XW], F32, tag="ps")
                        for ic in range(IC):
                            nc.tensor.matmul(
                                ps[:],
                                w_sb[:, ic * FG + fcg * 128: ic * FG + fcg * 128 + 128],
                                x_sb[:, ic * XW:(ic + 1) * XW],
                                start=(ic == 0),
                                stop=(not use_bias and ic == IC - 1),
                            )
                        if use_bias:
                            nc.tensor.matmul(
                                ps[:],
                                bq_sb[0:1, fc_abs * 128:(fc_abs + 1) * 128],
                                ones_row[0:1, 0:XW],
                                start=False,
                                stop=True,
                            )
                        nc.vector.tensor_mul(
                            qt_g[:, fcg * R + rb * XW: fcg * R + (rb + 1) * XW],
                            ps[:],
                            maskB[:, rb * XW:(rb + 1) * XW],
                        )

                # ---- K projection (epilogue on ScalarE: Act idle here) ----
                w_sb = load_w(wkT, f0, ppool)
                for rb in range(S // XW):
                    x_sb = load_x(kT, rb, ppool)
                    for fcg in range(FCG):
                        fc_abs = (f0 // 128) + fcg
                        ps = ppool.tile([128, XW], F32, tag="ps")
                        for ic in range(IC):
                            nc.tensor.matmul(
                                ps[:],
                                w_sb[:, ic * FG + fcg * 128: ic * FG + fcg * 128 + 128],
                                x_sb[:, ic * XW:(ic + 1) * XW],
                                start=(ic == 0),
                                stop=(not use_bias and ic == IC - 1),
                            )
                        if use_bias:
                            nc.tensor.matmul(
                                ps[:],
                                bk_sb[0:1, fc_abs * 128:(fc_abs + 1) * 128],
                                ones_row[0:1, 0:XW],
                                start=False,
                                stop=True,
                            )
                        nc.scalar.activation(
                            kt_g[:, fcg * S + rb * XW: fcg * S + (rb + 1) * XW],
                            ps[:],
                            AF.Copy,
                        )

                # ---- V projection + ones column ----
                w_sb = load_w(wvT, f0, ppool)
                NRC = XW // 128
                for rb4 in range(S // XW):
                    x_sb = load_x(vT, rb4, ppool)
                    for rcl in range(NRC):
                        rc = rb4 * NRC + rcl
                        ps = ppool.tile([128, FG], F32, tag="ps")
                        for ic in range(IC):
                            nc.tensor.matmul(
                                ps[:],
                                x_sb[:, ic * XW + rcl * 128: ic * XW + rcl * 128 + 128],
                                w_sb[:, ic * FG:(ic + 1) * FG],
                                start=(ic == 0),
                                stop=(not use_bias and ic == IC - 1),
                            )
                        if use_bias:
                            nc.tensor.matmul(
                                ps[:],
                                ones_row[0:1, 0:128],
                                bv_sb[0:1, f0:f0 + FG],
                                start=False,
                                stop=True,
                            )
                        # interleave the narrow per-head copies across
                        # ScalarE and DVE so neither gates the PE
                        for hl in range(HPG):
                            dst = vp_g[:, rc * WV + 65 * hl: rc * WV + 65 * hl + 64]
                            src = ps[:, hl * 64:(hl + 1) * 64]
                            if hl % 2 == 0:
                                nc.scalar.activation(dst, src, AF.Copy)
                            else:
                                nc.vector.tensor_copy(dst, src)

            if g == NG - 1:
                # prefetch the whole output-projection weight during the
                # last attention phase (SP queue is idle there)
                nc.sync.dma_start(
                    wo_all[:].rearrange("p (i f) -> p i f", i=IC),
                    woT[:, :].rearrange("(i p) f -> p i f", p=128),
                )

            # ---- attention: head pairs share the PE array via row groups ----
            with tc.tile_pool(name="sps", bufs=3, space="PSUM") as spool, \
                 tc.tile_pool(name="pvp", bufs=2, space="PSUM") as pvpool, \
                 tc.tile_pool(name="esb", bufs=3) as epool, \
                 tc.tile_pool(name="nrm", bufs=2) as npool, \
                 tc.tile_pool(name="pvs", bufs=2) as pvspool:
                for sqb in range(NSQB):
                    for hp in range(HPG // 2):
                        fcg = hp
                        q0 = qt_g[0:64,
                                  fcg * R + sqb * SQB: fcg * R + (sqb + 1) * SQB]
                        q1 = qt_g[64:128,
                                  fcg * R + sqb * SQB: fcg * R + (sqb + 1) * SQB]
                        pv0 = pvpool.tile([65, SQB], F32, tag="pv")
                        pv1 = pvpool.tile([65, SQB], F32, tag="pv")
                        pvs = [pv0, pv1]
                        for kch in range(NKC // 2):
                            sp0 = spool.tile([128, 2 * SQB], F32, tag="sp")
                            sp1 = spool.tile([128, 2 * SQB], F32, tag="sp")
                            sps = [sp0, sp1]
                            for j in range(2):
                                kc = 2 * kch + j
                                kslc = slice(fcg * S + kc * 128,
                                             fcg * S + kc * 128 + 128)
                                # heads 2hp (rows 0-63) and 2hp+1 (rows 64-127)
                                # run concurrently in disjoint PE row groups
                                nc.tensor.matmul(
                                    sps[0][:, j * SQB:(j + 1) * SQB],
                                    kt_g[0:64, kslc], q0, start=True, stop=True,
                                )
                                nc.tensor.matmul(
                                    sps[1][:, j * SQB:(j + 1) * SQB],
                                    kt_g[64:128, kslc], q1, start=True, stop=True,
                                )
                            # exp: even head exact on ScalarE; odd head
                            # Schraudolph fast-exp on DVE (Pool can't read
                            # PSUM on real HW)
                            es0 = epool.tile([128, 2 * SQB], BF16, tag="es")
                            nc.scalar.activation(es0[:], sps[0][:], AF.Exp)
                            if POOL_EXP:
                                es1i = epool.tile([128, 2 * SQB], I16, tag="esi")
                                nc.vector.tensor_scalar(
                                    es1i[:], sps[1][:], EXPA, EXPB,
                                    op0=ALU.mult, op1=ALU.add,
                                )
                                es1 = es1i[:].bitcast(BF16)
                            else:
                                es1b = epool.tile([128, 2 * SQB], BF16, tag="es")
                                nc.scalar.activation(es1b[:], sps[1][:], AF.Exp)
                                es1 = es1b[:]
                            ess = [es0[:], es1]
                            for j in range(2):
                                kc = 2 * kch + j
                                for h in range(2):
                                    hl = 2 * hp + h
                                    nc.tensor.matmul(
                                        pvs[h][:],
                                        vp_g[:, kc * WV + 65 * hl:
                                             kc * WV + 65 * hl + 65],
                                        ess[h][:, j * SQB:(j + 1) * SQB],
                                        start=(kc == 0),
                                        stop=(kc == NKC - 1),
                                    )
                        for h in range(2):
                            po = 64 * h
                            # copy PSUM->SBUF immediately to free the bank
                            # (on ScalarE), then normalize off the SBUF copy
                            # (reciprocal on DVE, final multiply on Pool)
                            pv_sb = pvspool.tile([65, SQB], F32, tag="pvsb")
                            nc.scalar.activation(pv_sb[:], pvs[h][:], AF.Copy)
                            recip = npool.tile([1, SQB], F32, tag="recip")
                            nc.vector.reciprocal(recip[:], pv_sb[64:65, :])
                            rd = dbounce.tile([1, SQB], F32, tag="rd")
                            nc.sync.dma_start(rd[:], recip[:])
                            recipB = npool.tile([64, SQB], F32, tag="recipB")
                            nc.sync.dma_start(
                                recipB[:], rd[:].to_broadcast([64, SQB])
                            )
                            nc.gpsimd.tensor_copy(
                                recipB[0:1, 0:1], recipB[0:1, 0:1]
                            )
                            fc_abs = (f0 // 128) + fcg
                            nc.gpsimd.tensor_mul(
                                h_tile[po:po + 64, fc_abs * R + sqb * SQB:
                                       fc_abs * R + (sqb + 1) * SQB],
                                pv_sb[0:64, :],
                                recipB[:],
                            )

        # ---- output projection (wo resident in SBUF, psum held across fc) ----
        OB = 512
        NOB = HID // OB
        R128 = R // 128
        NHALF = 2
        RPH = R128 // NHALF
        with tc.tile_pool(name="ops", bufs=RPH * NOB, space="PSUM") as opool, \
             tc.tile_pool(name="osb", bufs=2) as ospool:
            for half in range(NHALF):
                pss = []
                for _psi in range(RPH * NOB):
                    ps_acc = opool.tile([128, OB], F32, tag="ps")
                    pss.append(ps_acc)
                for fc in range(IC):
                    for ob in range(NOB):
                        for rl in range(RPH):
                            rc = half * RPH + rl
                            nc.tensor.matmul(
                                pss[rl * NOB + ob][:],
                                h_tile[:, fc * R + rc * 128: fc * R + rc * 128 + 128],
                                wo_all[:, fc * HID + ob * OB: fc * HID + (ob + 1) * OB],
                                start=(fc == 0),
                                stop=(not use_bias and fc == IC - 1),
                            )
                for rl in range(RPH):
                    rc = half * RPH + rl
                    for ob in range(NOB):
                        if use_bias:
                            nc.tensor.matmul(
                                pss[rl * NOB + ob][:],
                                ones_row[0:1, 0:128],
                                bo_sb[0:1, ob * OB:(ob + 1) * OB],
                                start=False,
                                stop=True,
                            )
                        o_sb = ospool.tile([128, OB], F32, tag="o")
                        nc.vector.tensor_copy(o_sb[:], pss[rl * NOB + ob][:])
                        nc.sync.dma_start(
                            out[rc * 128:(rc + 1) * 128, ob * OB:(ob + 1) * OB],
                            o_sb[:],
                        )
    return nc


_compiled = {}


def _get_nc(use_bias=False):
    key = (use_bias, POOL_EXP)
    if key not in _compiled:
        cfg = Cfg(use_bias=use_bias)
        nc = bacc.Bacc(
            "TRN2", target_bir_lowering=False, debug=False, num_devices=N_CORES
        )
        build(nc, cfg)
        nc.compile()
        _compiled[key] = (nc, cfg)
    return _compiled[key]


def _bf(x):
    return np.ascontiguousarray(x).astype(ml_dtypes.bfloat16)


def make_in_maps(q, k, v, mask, wq, bq, wk, bk, wv, bv, wo, bo, use_bias=False):
    q = np.asarray(q, dtype=np.float32)
    k = np.asarray(k, dtype=np.float32)
    v = np.asarray(v, dtype=np.float32)
    mask = np.asarray(mask)
    f32 = np.float32
    R = S // 2
    scale = f32(1.0 / np.sqrt(DH))

    shared = {
        "wqT": _bf(np.asarray(wq, f32).T),
        "wkT": _bf(np.asarray(wk, f32).T),
        "wvT": _bf(np.asarray(wv, f32).T),
        "woT": _bf(np.asarray(wo, f32).T),
    }
    if use_bias:
        shared["bqr"] = _bf(np.asarray(bq, f32).reshape(1, HID))
        shared["bkr"] = _bf(np.asarray(bk, f32).reshape(1, HID))
        shared["bvr"] = _bf(np.asarray(bv, f32).reshape(1, HID))
        shared["bor"] = _bf(np.asarray(bo, f32).reshape(1, HID))
    kT_b = [_bf(k[b].T) for b in range(B)]
    vT_b = [_bf(v[b].T) for b in range(B)]
    in_maps = []
    for c in range(N_CORES):
        b, half = c // 2, c % 2
        rows = slice(half * R, (half + 1) * R)
        m = dict(shared)
        m["qT"] = _bf(q[b, rows].T)
        m["kT"] = kT_b[b]
        m["vT"] = vT_b[b]
        m["maskf"] = ((mask[b, rows] != 0).astype(f32) * scale).reshape(1, R)
        in_maps.append(m)
    return in_maps


def kernel(q, k, v, mask, wq, bq, wk, bk, wv, bv, wo, bo):
    use_bias = bool(
        np.any(np.asarray(bq)) or np.any(np.asarray(bk))
        or np.any(np.asarray(bv)) or np.any(np.asarray(bo))
    )
    nc, cfg = _get_nc(use_bias)
    in_maps = make_in_maps(q, k, v, mask, wq, bq, wk, bk, wv, bv, wo, bo,
                           use_bias=use_bias)

    res = run_bass_kernel_spmd(nc, in_maps, list(range(N_CORES)), trace=TRACE)
    LAST_RESULTS[0] = res

    R = S // 2
    out = np.empty((B, S, HID), dtype=np.float32)
    for c in range(N_CORES):
        b, half = c // 2, c % 2
        out[b, half * R:(half + 1) * R, :] = res.results[c]["out"]
    return out
